# revision 5
# baseline (speedup 1.0000x reference)
"""RBF kernel matrix on 8 Trainium2 NeuronCores.

K[i, j] = exp(-gamma * ||x_i - y_j||^2),  x: (8192, 64), y: (8192, 64).

Strategy: shard rows of x across the 8 cores (1024 rows each), replicate y.

Numerics ("consistent rounding"): round x, y to fp16 ON THE HOST and compute
ALL terms of the expansion ||x-y||^2 = ||x||^2 + ||y||^2 - 2 x.y from the
SAME rounded vectors.  The device then computes exactly

    arg = 2*gamma * ( x_h . y_h  -  ||y_h||^2/2 )  -  gamma*||x_h||^2
        = -gamma * || x_h - y_h ||^2

i.e. the true RBF argument for the perturbed points (x_h, y_h).  The output
error is then  2*gamma*(dx - dy).(x - y), which vanishes exactly where the
kernel peaks (x ~ y), so a SINGLE fp16 matmul pass suffices (measured
rel_max ~ 6e-4 vs the 2e-2 gate).  Layout per core:

    rows  0..63   x_h^T (fp16)          vs  y_h^T (fp16)
    rows 64..65   ones                  vs  hi/lo fp16 of -||y_h||^2/2

one K=66 matmul per 512-col PSUM bank (single streaming pass - PE stays off
the critical path even at the mid p-state).  The ScalarE activation applies
Exp(psum*scale + bias) with scale = 2*gamma and bias = -gamma*||x_h||^2 as
per-partition fp32 APs (both runtime data - nothing about gamma is baked
into the NEFF).  Output is written bf16 (halves the HBM write traffic vs
fp32; adds ~2e-3 rounding, still far under the gate) and upcast to fp32 on
the host.

Schedule notes (from perfetto traces): ScalarE's Exp is the bottleneck
engine (~2.02us per 128x2048 tile, 32 tiles); input DMAs are ordered
xs -> ys[0:1024] -> bias/scale -> rest of ys so the pipeline fills ~5us
sooner; eight dummy warmup matmuls ramp the PE p-state while inputs load;
the first and last tiles are activated/DMA'd in 1024-col halves to shorten
pipeline fill and drain.
"""

import numpy as np

from concourse import bacc, tile, mybir
from concourse.bass_utils import run_bass_kernel_spmd

N_CORES = 8
BX, BY, F = 8192, 8192, 64
M_CORE = BX // N_CORES      # 1024 rows of x per core
K = F + 2                   # 64 features + 2 rows for -||y||^2/2 hi/lo
MM_N = 512                  # one PSUM bank of fp32
GRP = 4                     # PSUM banks per ACT/DMA tile
GRP_N = MM_N * GRP          # 2048 columns per ACT/DMA tile
N_MI = M_CORE // 128        # 8 row chunks
N_NI = BY // GRP_N          # 4 column groups
YS_CHUNKS = (1024, 1024, 2048, 4096)   # staged y-load column chunks

_cache: dict = {}


def _build():
    if "nc" in _cache:
        return _cache["nc"]

    f32 = mybir.dt.float32
    f16 = mybir.dt.float16
    bf16 = mybir.dt.bfloat16
    Exp = mybir.ActivationFunctionType.Exp
    nc = bacc.Bacc(None, target_bir_lowering=False, debug=False)
    xs = nc.dram_tensor("xs", (K, M_CORE), f16, kind="ExternalInput")
    ys = nc.dram_tensor("ys", (K, BY), f16, kind="ExternalInput")
    xqg = nc.dram_tensor("xqg", (128, N_MI + 1), f32, kind="ExternalInput")
    out = nc.dram_tensor("out", (M_CORE, BY), bf16, kind="ExternalOutput")

    with tile.TileContext(nc) as tc:
        with (
            tc.tile_pool(name="const", bufs=1) as cpool,
            tc.tile_pool(name="obuf", bufs=4) as opool,
            tc.tile_pool(name="psum", bufs=2, space="PSUM") as ppool,
        ):
            xs_sb = cpool.tile((K, M_CORE), f16)
            ys_sb = cpool.tile((K, BY), f16)
            xqg_sb = cpool.tile((128, N_MI + 1), f32)
            wtmp = cpool.tile((K, MM_N), f16)

            # input DMAs, most-urgent first (sync queue is serial)
            nc.sync.dma_start(out=xs_sb[:], in_=xs[:])
            c0 = 0
            for i, w in enumerate(YS_CHUNKS):
                nc.sync.dma_start(
                    out=ys_sb[:, c0 : c0 + w], in_=ys[:, c0 : c0 + w]
                )
                if i == 0:
                    nc.sync.dma_start(out=xqg_sb[:], in_=xqg[:])
                c0 += w

            nc.vector.memset(wtmp[:], 0.0)

            bias = lambda mi: xqg_sb[:, mi : mi + 1]
            scale = xqg_sb[:, N_MI : N_MI + 1]

            for mi in range(N_MI):
                w = xs_sb[:, mi * 128 : (mi + 1) * 128]
                for ni in range(N_NI):
                    ps = ppool.tile((128, GRP_N), f32)
                    if mi == 0 and ni == 0:
                        # warm up the PE p-state while inputs stream in:
                        # dummy matmuls, overwritten by the real start=True
                        # matmul below
                        for _ in range(8):
                            nc.tensor.matmul(
                                ps[:, 0:MM_N],
                                wtmp[:, 0:128],
                                wtmp[:],
                                start=True,
                                stop=True,
                            )
                    for j in range(GRP):
                        c0 = ni * GRP_N + j * MM_N
                        nc.tensor.matmul(
                            ps[:, j * MM_N : (j + 1) * MM_N],
                            w,
                            ys_sb[:, c0 : c0 + MM_N],
                            start=True,
                            stop=True,
                        )
                    first = mi == 0 and ni == 0
                    last = mi == N_MI - 1 and ni == N_NI - 1
                    if first or last:
                        # halve the first tile (starts draining sooner) and
                        # the last tile (shorter pipeline drain)
                        for h in range(2):
                            oth = opool.tile((128, GRP_N // 2), bf16)
                            sl = slice(h * GRP_N // 2, (h + 1) * GRP_N // 2)
                            nc.scalar.activation(
                                oth[:], ps[:, sl], Exp,
                                bias=bias(mi), scale=scale,
                            )
                            nc.sync.dma_start(
                                out=out[
                                    mi * 128 : (mi + 1) * 128,
                                    ni * GRP_N + h * GRP_N // 2
                                    : ni * GRP_N + (h + 1) * GRP_N // 2,
                                ],
                                in_=oth[:],
                            )
                    else:
                        ot = opool.tile((128, GRP_N), bf16)
                        nc.scalar.activation(
                            ot[:], ps[:], Exp, bias=bias(mi), scale=scale
                        )
                        nc.sync.dma_start(
                            out=out[
                                mi * 128 : (mi + 1) * 128,
                                ni * GRP_N : (ni + 1) * GRP_N,
                            ],
                            in_=ot[:],
                        )

    nc.compile()
    _cache["nc"] = nc
    return nc


def _prep_inputs(x, y, gamma):
    x = np.ascontiguousarray(np.asarray(x, dtype=np.float32))
    y = np.ascontiguousarray(np.asarray(y, dtype=np.float32))
    g = np.float64(np.asarray(gamma, dtype=np.float32))

    xh = x.astype(np.float16)                       # rounded x
    yh = y.astype(np.float16)                       # rounded y
    xsq = (xh.astype(np.float64) ** 2).sum(axis=1)  # ||x_h||^2 (exact-ish)
    ysq = (yh.astype(np.float64) ** 2).sum(axis=1)

    ones = np.ones((2, M_CORE), dtype=np.float16)
    yqv = -0.5 * ysq                                # scale 2*gamma applied later
    yq1 = yqv.astype(np.float16)
    yq2 = (yqv - yq1.astype(np.float64)).astype(np.float16)
    ys_full = np.concatenate([yh.T, yq1[None, :], yq2[None, :]], axis=0)
    ys_full = np.ascontiguousarray(ys_full)         # (66, 8192) fp16

    xq_full = (-g * xsq).astype(np.float32)         # (8192,) bias rows

    xs_cores, xqg_cores = [], []
    for c in range(N_CORES):
        sl = slice(c * M_CORE, (c + 1) * M_CORE)
        xs_c = np.concatenate([xh[sl].T, ones], axis=0)
        xs_cores.append(np.ascontiguousarray(xs_c))             # (66, 1024)
        xqg_c = np.empty((128, N_MI + 1), dtype=np.float32)
        xqg_c[:, :N_MI] = xq_full[sl].reshape(N_MI, 128).T      # bias columns
        xqg_c[:, N_MI] = np.float32(2.0 * g)                    # ACT scale
        xqg_cores.append(np.ascontiguousarray(xqg_c))
    return xs_cores, ys_full, xqg_cores


def _run(x, y, gamma, trace=False, tmpdir=None):
    nc = _build()
    xs_cores, ys_full, xqg_cores = _prep_inputs(x, y, gamma)
    in_maps = [
        {"xs": xs_cores[c], "ys": ys_full, "xqg": xqg_cores[c]}
        for c in range(N_CORES)
    ]
    res = run_bass_kernel_spmd(
        nc, in_maps, list(range(N_CORES)), trace=trace, tmpdir=tmpdir
    )
    full = np.concatenate(
        [np.asarray(res.results[c]["out"]) for c in range(N_CORES)], axis=0
    )
    return full.astype(np.float32), res


def kernel(x, y, gamma):
    full, _ = _run(x, y, gamma, trace=False)
    return full


def kernel_traced(x, y, gamma, tmpdir=None):
    """test.py helper: returns (output, BassKernelResults with profile)."""
    return _run(x, y, gamma, trace=True, tmpdir=tmpdir)


# revision 8
# speedup vs baseline: 1.0242x; 1.0242x over previous
"""RBF kernel matrix on 8 Trainium2 NeuronCores.

K[i, j] = exp(-gamma * ||x_i - y_j||^2),  x: (8192, 64), y: (8192, 64).

Strategy: shard rows of x across the 8 cores (1024 rows each), replicate y.

Numerics ("consistent rounding"): round x, y to fp16 ON THE HOST and compute
ALL terms of the expansion ||x-y||^2 = ||x||^2 + ||y||^2 - 2 x.y from the
SAME rounded vectors.  The device then computes exactly

    arg = 2*gamma * ( x_h . y_h  -  ||y_h||^2/2 )  -  gamma*||x_h||^2
        = -gamma * || x_h - y_h ||^2

i.e. the true RBF argument for the perturbed points (x_h, y_h).  The output
error is then  2*gamma*(dx - dy).(x - y), which vanishes exactly where the
kernel peaks (x ~ y), so a SINGLE fp16 matmul pass suffices (measured
rel_max ~ 6e-4 vs the 2e-2 gate).  Layout per core:

    rows  0..63   x_h^T (fp16)          vs  y_h^T (fp16)
    rows 64..65   ones                  vs  hi/lo fp16 of -||y_h||^2/2

one K=66 matmul per 512-col PSUM bank (single streaming pass - PE stays off
the critical path even at the mid p-state).  The ScalarE activation applies
Exp(psum*scale + bias) with scale = 2*gamma and bias = -gamma*||x_h||^2 as
per-partition fp32 APs (both runtime data - nothing about gamma is baked
into the NEFF).  Output is written bf16 (halves the HBM write traffic vs
fp32; adds ~2e-3 rounding, still far under the gate) and upcast to fp32 on
the host.

Schedule notes (from perfetto traces): ScalarE's Exp is the bottleneck
engine (~2.02us per 128x2048 tile, 32 tiles); input DMAs are ordered
xs -> ys[0:1024] -> bias/scale -> rest of ys so the pipeline fills ~5us
sooner; eight dummy warmup matmuls ramp the PE p-state while inputs load;
the first and last tiles are activated/DMA'd in 1024-col halves to shorten
pipeline fill and drain.
"""

import numpy as np

from concourse import bacc, tile, mybir
from concourse.bass_utils import run_bass_kernel_spmd

N_CORES = 8
BX, BY, F = 8192, 8192, 64
M_CORE = BX // N_CORES      # 1024 rows of x per core
K = F + 2                   # 64 features + 2 rows for -||y||^2/2 hi/lo
MM_N = 512                  # one PSUM bank of fp32
GRP = 4                     # PSUM banks per ACT/DMA tile
GRP_N = MM_N * GRP          # 2048 columns per ACT/DMA tile
N_MI = M_CORE // 128        # 8 row chunks
N_NI = BY // GRP_N          # 4 column groups

_cache: dict = {}


def _build():
    if "nc" in _cache:
        return _cache["nc"]

    f32 = mybir.dt.float32
    f16 = mybir.dt.float16
    bf16 = mybir.dt.bfloat16
    Exp = mybir.ActivationFunctionType.Exp
    nc = bacc.Bacc(None, target_bir_lowering=False, debug=False)
    xs = nc.dram_tensor("xs", (K, M_CORE), f16, kind="ExternalInput")
    ys = nc.dram_tensor("ys", (K, BY), f16, kind="ExternalInput")
    xqg = nc.dram_tensor("xqg", (128, N_MI + 1), f32, kind="ExternalInput")
    out = nc.dram_tensor("out", (M_CORE, BY), bf16, kind="ExternalOutput")

    with tile.TileContext(nc) as tc:
        with (
            tc.tile_pool(name="const", bufs=1) as cpool,
            tc.tile_pool(name="obuf", bufs=4) as opool,
            tc.tile_pool(name="psum", bufs=2, space="PSUM") as ppool,
        ):
            xs_sb = cpool.tile((K, M_CORE), f16)
            ys_sb = cpool.tile((K, BY), f16)
            xqg_sb = cpool.tile((128, N_MI + 1), f32)
            wtmp = cpool.tile((K, MM_N), f16)

            # input DMAs, most-urgent first; y columns 2048:4096 go down the
            # (otherwise idle until ~13us) scalar engine's DMA queue so the
            # two hardware queues stream y in parallel
            nc.sync.dma_start(out=ys_sb[:, 0:2048], in_=ys[:, 0:2048])
            nc.scalar.dma_start(out=ys_sb[:, 2048:4096], in_=ys[:, 2048:4096])
            nc.sync.dma_start(out=xs_sb[:], in_=xs[:])
            nc.sync.dma_start(out=xqg_sb[:], in_=xqg[:])
            nc.sync.dma_start(out=ys_sb[:, 4096:BY], in_=ys[:, 4096:BY])

            nc.vector.memset(wtmp[:], 0.0)

            bias = lambda mi: xqg_sb[:, mi : mi + 1]
            scale = xqg_sb[:, N_MI : N_MI + 1]

            for mi in range(N_MI):
                w = xs_sb[:, mi * 128 : (mi + 1) * 128]
                for ni in range(N_NI):
                    ps = ppool.tile((128, GRP_N), f32)
                    if mi == 0 and ni == 0:
                        # warm up the PE p-state while inputs stream in:
                        # dummy matmuls, overwritten by the real start=True
                        # matmul below
                        for _ in range(8):
                            nc.tensor.matmul(
                                ps[:, 0:MM_N],
                                wtmp[:, 0:128],
                                wtmp[:],
                                start=True,
                                stop=True,
                            )
                    for j in range(GRP):
                        c0 = ni * GRP_N + j * MM_N
                        nc.tensor.matmul(
                            ps[:, j * MM_N : (j + 1) * MM_N],
                            w,
                            ys_sb[:, c0 : c0 + MM_N],
                            start=True,
                            stop=True,
                        )
                    last = mi == N_MI - 1 and ni == N_NI - 1
                    if last:
                        # halve the last tile (shorter pipeline drain)
                        for h in range(2):
                            oth = opool.tile((128, GRP_N // 2), bf16)
                            sl = slice(h * GRP_N // 2, (h + 1) * GRP_N // 2)
                            nc.scalar.activation(
                                oth[:], ps[:, sl], Exp,
                                bias=bias(mi), scale=scale,
                            )
                            nc.sync.dma_start(
                                out=out[
                                    mi * 128 : (mi + 1) * 128,
                                    ni * GRP_N + h * GRP_N // 2
                                    : ni * GRP_N + (h + 1) * GRP_N // 2,
                                ],
                                in_=oth[:],
                            )
                    else:
                        ot = opool.tile((128, GRP_N), bf16)
                        nc.scalar.activation(
                            ot[:], ps[:], Exp, bias=bias(mi), scale=scale
                        )
                        nc.sync.dma_start(
                            out=out[
                                mi * 128 : (mi + 1) * 128,
                                ni * GRP_N : (ni + 1) * GRP_N,
                            ],
                            in_=ot[:],
                        )

    nc.compile()
    _cache["nc"] = nc
    return nc


def _prep_inputs(x, y, gamma):
    x = np.ascontiguousarray(np.asarray(x, dtype=np.float32))
    y = np.ascontiguousarray(np.asarray(y, dtype=np.float32))
    g = np.float64(np.asarray(gamma, dtype=np.float32))

    xh = x.astype(np.float16)                       # rounded x
    yh = y.astype(np.float16)                       # rounded y
    xsq = (xh.astype(np.float64) ** 2).sum(axis=1)  # ||x_h||^2 (exact-ish)
    ysq = (yh.astype(np.float64) ** 2).sum(axis=1)

    ones = np.ones((2, M_CORE), dtype=np.float16)
    yqv = -0.5 * ysq                                # scale 2*gamma applied later
    yq1 = yqv.astype(np.float16)
    yq2 = (yqv - yq1.astype(np.float64)).astype(np.float16)
    ys_full = np.concatenate([yh.T, yq1[None, :], yq2[None, :]], axis=0)
    ys_full = np.ascontiguousarray(ys_full)         # (66, 8192) fp16

    xq_full = (-g * xsq).astype(np.float32)         # (8192,) bias rows

    xs_cores, xqg_cores = [], []
    for c in range(N_CORES):
        sl = slice(c * M_CORE, (c + 1) * M_CORE)
        xs_c = np.concatenate([xh[sl].T, ones], axis=0)
        xs_cores.append(np.ascontiguousarray(xs_c))             # (66, 1024)
        xqg_c = np.empty((128, N_MI + 1), dtype=np.float32)
        xqg_c[:, :N_MI] = xq_full[sl].reshape(N_MI, 128).T      # bias columns
        xqg_c[:, N_MI] = np.float32(2.0 * g)                    # ACT scale
        xqg_cores.append(np.ascontiguousarray(xqg_c))
    return xs_cores, ys_full, xqg_cores


def _run(x, y, gamma, trace=False, tmpdir=None):
    nc = _build()
    xs_cores, ys_full, xqg_cores = _prep_inputs(x, y, gamma)
    in_maps = [
        {"xs": xs_cores[c], "ys": ys_full, "xqg": xqg_cores[c]}
        for c in range(N_CORES)
    ]
    res = run_bass_kernel_spmd(
        nc, in_maps, list(range(N_CORES)), trace=trace, tmpdir=tmpdir
    )
    full = np.concatenate(
        [np.asarray(res.results[c]["out"]) for c in range(N_CORES)], axis=0
    )
    return full.astype(np.float32), res


def kernel(x, y, gamma):
    full, _ = _run(x, y, gamma, trace=False)
    return full


def kernel_traced(x, y, gamma, tmpdir=None):
    """test.py helper: returns (output, BassKernelResults with profile)."""
    return _run(x, y, gamma, trace=True, tmpdir=tmpdir)


# revision 9
# speedup vs baseline: 1.0369x; 1.0125x over previous
"""RBF kernel matrix on 8 Trainium2 NeuronCores.

K[i, j] = exp(-gamma * ||x_i - y_j||^2),  x: (8192, 64), y: (8192, 64).

Strategy: 4x2 core grid - core c computes x-row block (c//2) x y-column
half (c%2), i.e. a (2048, 4096) output block.  This halves the per-core
input bytes vs pure row sharding (input DMAs stream at only ~50-90 GB/s
per queue, so the replicated-y load otherwise gates pipeline fill).

Numerics ("consistent rounding"): round x, y to fp16 ON THE HOST and compute
ALL terms of the expansion ||x-y||^2 = ||x||^2 + ||y||^2 - 2 x.y from the
SAME rounded vectors.  The device then computes exactly

    arg = 2*gamma * ( x_h . y_h  -  ||y_h||^2/2 )  -  gamma*||x_h||^2
        = -gamma * || x_h - y_h ||^2

i.e. the true RBF argument for the perturbed points (x_h, y_h).  The output
error is then  2*gamma*(dx - dy).(x - y), which vanishes exactly where the
kernel peaks (x ~ y), so a SINGLE fp16 matmul pass suffices (measured
rel_max ~ 6e-4 vs the 2e-2 gate).  Layout per core:

    rows  0..63   x_h^T (fp16)          vs  y_h^T (fp16)
    rows 64..65   ones                  vs  hi/lo fp16 of -||y_h||^2/2

one K=66 matmul per 512-col PSUM bank (single streaming pass - PE stays off
the critical path even at the mid p-state).  The ScalarE activation applies
Exp(psum*scale + bias) with scale = 2*gamma and bias = -gamma*||x_h||^2 as
per-partition fp32 APs (both runtime data - nothing about gamma is baked
into the NEFF).  Output is written bf16 (halves the HBM write traffic vs
fp32; adds ~2e-3 rounding, still far under the gate) and upcast to fp32 on
the host.

Schedule notes (from perfetto traces): ScalarE's Exp is the bottleneck
engine (~2.0us per 128x2048 tile, 32 tiles, zero steady-state gaps); the
two y column stripes stream down the two hardware DMA queues (sync + the
otherwise-idle scalar engine queue) in parallel; eight dummy warmup matmuls
ramp the PE p-state while inputs load; the last tile is activated/DMA'd in
1024-col halves to shorten the pipeline drain.
"""

import numpy as np

from concourse import bacc, tile, mybir
from concourse.bass_utils import run_bass_kernel_spmd

N_CORES = 8
BX, BY, F = 8192, 8192, 64
R_BLOCKS, C_BLOCKS = 4, 2   # core grid: 4 x-row blocks x 2 y-column halves
M_CORE = BX // R_BLOCKS     # 2048 rows of x per core
B_CORE = BY // C_BLOCKS     # 4096 columns of y per core
K = F + 2                   # 64 features + 2 rows for -||y||^2/2 hi/lo
MM_N = 512                  # one PSUM bank of fp32
GRP = 4                     # PSUM banks per ACT/DMA tile
GRP_N = MM_N * GRP          # 2048 columns per ACT/DMA tile
N_MI = M_CORE // 128        # 16 row chunks
N_NI = B_CORE // GRP_N      # 2 column groups

_cache: dict = {}


def _build():
    if "nc" in _cache:
        return _cache["nc"]

    f32 = mybir.dt.float32
    f16 = mybir.dt.float16
    bf16 = mybir.dt.bfloat16
    Exp = mybir.ActivationFunctionType.Exp
    nc = bacc.Bacc(None, target_bir_lowering=False, debug=False)
    xs = nc.dram_tensor("xs", (K, M_CORE), f16, kind="ExternalInput")
    ys = nc.dram_tensor("ys", (K, B_CORE), f16, kind="ExternalInput")
    xqg = nc.dram_tensor("xqg", (128, N_MI + 1), f32, kind="ExternalInput")
    out = nc.dram_tensor("out", (M_CORE, B_CORE), bf16, kind="ExternalOutput")

    with tile.TileContext(nc) as tc:
        with (
            tc.tile_pool(name="const", bufs=1) as cpool,
            tc.tile_pool(name="obuf", bufs=4) as opool,
            tc.tile_pool(name="psum", bufs=2, space="PSUM") as ppool,
        ):
            xs_sb = cpool.tile((K, M_CORE), f16)
            ys_sb = cpool.tile((K, B_CORE), f16)
            xqg_sb = cpool.tile((128, N_MI + 1), f32)
            wtmp = cpool.tile((K, MM_N), f16)

            # parallel input streams: sync queue feeds the first y stripe +
            # bias/scale, the (otherwise idle until ~12us) scalar engine's
            # queue feeds x + the second y stripe
            nc.sync.dma_start(out=ys_sb[:, 0:GRP_N], in_=ys[:, 0:GRP_N])
            nc.scalar.dma_start(out=xs_sb[:], in_=xs[:])
            nc.sync.dma_start(out=xqg_sb[:], in_=xqg[:])
            nc.scalar.dma_start(
                out=ys_sb[:, GRP_N:B_CORE], in_=ys[:, GRP_N:B_CORE]
            )

            nc.vector.memset(wtmp[:], 0.0)

            bias = lambda mi: xqg_sb[:, mi : mi + 1]
            scale = xqg_sb[:, N_MI : N_MI + 1]

            for mi in range(N_MI):
                w = xs_sb[:, mi * 128 : (mi + 1) * 128]
                for ni in range(N_NI):
                    ps = ppool.tile((128, GRP_N), f32)
                    if mi == 0 and ni == 0:
                        # warm up the PE p-state while inputs stream in:
                        # dummy matmuls, overwritten by the real start=True
                        # matmul below
                        for _ in range(8):
                            nc.tensor.matmul(
                                ps[:, 0:MM_N],
                                wtmp[:, 0:128],
                                wtmp[:],
                                start=True,
                                stop=True,
                            )
                    for j in range(GRP):
                        c0 = ni * GRP_N + j * MM_N
                        nc.tensor.matmul(
                            ps[:, j * MM_N : (j + 1) * MM_N],
                            w,
                            ys_sb[:, c0 : c0 + MM_N],
                            start=True,
                            stop=True,
                        )
                    last = mi == N_MI - 1 and ni == N_NI - 1
                    if last:
                        # halve the last tile (shorter pipeline drain)
                        for h in range(2):
                            oth = opool.tile((128, GRP_N // 2), bf16)
                            sl = slice(h * GRP_N // 2, (h + 1) * GRP_N // 2)
                            nc.scalar.activation(
                                oth[:], ps[:, sl], Exp,
                                bias=bias(mi), scale=scale,
                            )
                            nc.sync.dma_start(
                                out=out[
                                    mi * 128 : (mi + 1) * 128,
                                    ni * GRP_N + h * GRP_N // 2
                                    : ni * GRP_N + (h + 1) * GRP_N // 2,
                                ],
                                in_=oth[:],
                            )
                    else:
                        ot = opool.tile((128, GRP_N), bf16)
                        nc.scalar.activation(
                            ot[:], ps[:], Exp, bias=bias(mi), scale=scale
                        )
                        nc.sync.dma_start(
                            out=out[
                                mi * 128 : (mi + 1) * 128,
                                ni * GRP_N : (ni + 1) * GRP_N,
                            ],
                            in_=ot[:],
                        )

    nc.compile()
    _cache["nc"] = nc
    return nc


def _prep_inputs(x, y, gamma):
    x = np.ascontiguousarray(np.asarray(x, dtype=np.float32))
    y = np.ascontiguousarray(np.asarray(y, dtype=np.float32))
    g = np.float64(np.asarray(gamma, dtype=np.float32))

    xh = x.astype(np.float16)                       # rounded x
    yh = y.astype(np.float16)                       # rounded y
    xsq = (xh.astype(np.float64) ** 2).sum(axis=1)  # ||x_h||^2 (exact-ish)
    ysq = (yh.astype(np.float64) ** 2).sum(axis=1)

    ones = np.ones((2, M_CORE), dtype=np.float16)
    yqv = -0.5 * ysq                                # scale 2*gamma applied later
    yq1 = yqv.astype(np.float16)
    yq2 = (yqv - yq1.astype(np.float64)).astype(np.float16)
    ys_all = np.concatenate([yh.T, yq1[None, :], yq2[None, :]], axis=0)

    xq_full = (-g * xsq).astype(np.float32)         # (8192,) bias rows

    xs_blocks, xqg_blocks = [], []
    for r in range(R_BLOCKS):
        sl = slice(r * M_CORE, (r + 1) * M_CORE)
        xs_r = np.concatenate([xh[sl].T, ones], axis=0)
        xs_blocks.append(np.ascontiguousarray(xs_r))            # (66, 2048)
        xqg_r = np.empty((128, N_MI + 1), dtype=np.float32)
        xqg_r[:, :N_MI] = xq_full[sl].reshape(N_MI, 128).T      # bias columns
        xqg_r[:, N_MI] = np.float32(2.0 * g)                    # ACT scale
        xqg_blocks.append(xqg_r)
    ys_halves = [
        np.ascontiguousarray(ys_all[:, h * B_CORE : (h + 1) * B_CORE])
        for h in range(C_BLOCKS)
    ]
    return xs_blocks, ys_halves, xqg_blocks


def _run(x, y, gamma, trace=False, tmpdir=None):
    nc = _build()
    xs_blocks, ys_halves, xqg_blocks = _prep_inputs(x, y, gamma)
    in_maps = [
        {
            "xs": xs_blocks[c // C_BLOCKS],
            "ys": ys_halves[c % C_BLOCKS],
            "xqg": xqg_blocks[c // C_BLOCKS],
        }
        for c in range(N_CORES)
    ]
    res = run_bass_kernel_spmd(
        nc, in_maps, list(range(N_CORES)), trace=trace, tmpdir=tmpdir
    )
    full = np.empty((BX, BY), dtype=np.float32)
    for c in range(N_CORES):
        r, h = c // C_BLOCKS, c % C_BLOCKS
        full[
            r * M_CORE : (r + 1) * M_CORE, h * B_CORE : (h + 1) * B_CORE
        ] = np.asarray(res.results[c]["out"]).astype(np.float32)
    return full, res


def kernel(x, y, gamma):
    full, _ = _run(x, y, gamma, trace=False)
    return full


def kernel_traced(x, y, gamma, tmpdir=None):
    """test.py helper: returns (output, BassKernelResults with profile)."""
    return _run(x, y, gamma, trace=True, tmpdir=tmpdir)
